# revision 34
# baseline (speedup 1.0000x reference)
"""Trainium2 Bass kernel for nn_BRIDGEDecoder (gnn_message_passing).

Strategy: data-parallel over the 16 graphs, 2 graphs per NeuronCore.
Host does index preprocessing only (dense adjacency + in-degree histograms
via bincount, weight layout prep); the device does all float NN compute:
  h1 = leaky(x @ W21.T + b21)            (per-graph [1,1024] matmul)
  agg = (adj.T @ h) / deg                (dense matmuls against SBUF adjacency)
  h2 = leaky([Wrel1|Wroot1|b] @ [t1;h1;1])   (packed K=3 matmul, channel-major)
  out = leaky([Wrel2|Wroot2|b] @ [t2;h2;1])  (packed K=17 matmul)
  k = (out/pmax).T @ out per n-tile      (PE, K=16, bf16)
  conn = sigmoid(pmax * (ln(adj+1e-10) + k/pmax))   with pmax = max(p, 1e-6):
         the per-graph p multiply is folded into the Sigmoid's per-partition
         scale AP, and 1/pmax into the k matmul's stationary operand.
  param = relu(x @ W11.T + b11)          (f32 path)

bf16 is used for the adjacency stream and matmul operands (PE runs 1 cyc/col
at bf16 vs 2 at fp32, and adjacency DMA halves); accumulation stays fp32 in
PSUM, and the sigmoid chain runs fp32. The param path is kept fp32.
Emit order is per-engine execution order: conv(0) first, then k(0) units
interleaved with conv(1) stages (graph 1's leaky runs on DVE so the ACT
queue during the k phase stays pure Sigmoid), then k(1).
"""

import sys

if "/opt/trn_rl_repo" not in sys.path:
    sys.path.insert(0, "/opt/trn_rl_repo")

import numpy as np

B, N, F = 16, 1024, 256
P = 128
NT = N // P  # 8 n-tiles per graph
G_PER_CORE = 2
N_CORES = 8
EPS = 1e-10
NEG = 0.01

_CACHE = {}


def _build_nc():
    import concourse.mybir as mybir
    import concourse.tile as tile
    from concourse import bacc
    from concourse.bass import ds, ts
    from concourse.masks import make_identity

    f32 = mybir.dt.float32
    bf16 = mybir.dt.bfloat16
    AF = mybir.ActivationFunctionType
    OP = mybir.AluOpType

    nc = bacc.Bacc("TRN2", target_bir_lowering=False, debug=False)

    adj_d = nc.dram_tensor("adj", [G_PER_CORE, N, N], bf16, kind="ExternalInput")
    xta_d = nc.dram_tensor("xta", [F + 1, G_PER_CORE], f32, kind="ExternalInput")
    xtab_d = nc.dram_tensor("xtab", [F + 1, G_PER_CORE], bf16, kind="ExternalInput")
    w21tb_d = nc.dram_tensor("w21tb", [F + 1, N], bf16, kind="ExternalInput")
    w11tb_d = nc.dram_tensor("w11tb", [F + 1, 1], f32, kind="ExternalInput")
    lhs1_d = nc.dram_tensor("lhs1", [3, 8], bf16, kind="ExternalInput")
    lhs2_d = nc.dram_tensor("lhs2", [17, 16], bf16, kind="ExternalInput")
    invdeg_d = nc.dram_tensor("invdeg", [G_PER_CORE, 8, N], f32, kind="ExternalInput")

    conn_d = nc.dram_tensor("conn", [G_PER_CORE, N, N], f32, kind="ExternalOutput")
    param_d = nc.dram_tensor("param", [1, G_PER_CORE], f32, kind="ExternalOutput")

    with tile.TileContext(nc) as tc:
        with (
            tc.tile_pool(name="consts", bufs=1) as consts,
            tc.tile_pool(name="adjp", bufs=2) as adjp,
            tc.tile_pool(name="work", bufs=2) as work,
            tc.tile_pool(name="ew", bufs=3) as ewp,
            tc.tile_pool(name="up", bufs=17) as up,
            tc.tile_pool(name="outp", bufs=3) as outp,
            tc.tile_pool(name="ps_big", bufs=3, space="PSUM") as ps_big,
            tc.tile_pool(name="ps_k", bufs=2, space="PSUM") as ps_k,
        ):
            # ---------------- constants (small tensors first: param path) ----
            xta0 = consts.tile([P, G_PER_CORE], f32)
            nc.sync.dma_start(xta0[:], xta_d[0:P])
            xta1 = consts.tile([P, G_PER_CORE], f32)
            nc.sync.dma_start(xta1[:], xta_d[P : 2 * P])
            xta2 = consts.tile([1, G_PER_CORE], f32)
            nc.sync.dma_start(xta2[:], xta_d[2 * P : 2 * P + 1])

            w11t0 = consts.tile([P, 1], f32)
            nc.sync.dma_start(w11t0[:], w11tb_d[0:P])
            w11t1 = consts.tile([P, 1], f32)
            nc.sync.dma_start(w11t1[:], w11tb_d[P : 2 * P])
            w11t2 = consts.tile([1, 1], f32)
            nc.sync.dma_start(w11t2[:], w11tb_d[2 * P : 2 * P + 1])

            xtb0 = consts.tile([P, G_PER_CORE], bf16)
            nc.sync.dma_start(xtb0[:], xtab_d[0:P])
            xtb1 = consts.tile([P, G_PER_CORE], bf16)
            nc.sync.dma_start(xtb1[:], xtab_d[P : 2 * P])
            xtb2 = consts.tile([1, G_PER_CORE], bf16)
            nc.sync.dma_start(xtb2[:], xtab_d[2 * P : 2 * P + 1])

            lhs1_t = consts.tile([3, 8], bf16)  # [Wrel1; Wroot1; brel1]
            nc.sync.dma_start(lhs1_t[:], lhs1_d[:])
            lhs2_t = consts.tile([17, 16], bf16)  # [Wrel2.T; Wroot2.T; brel2]
            nc.sync.dma_start(lhs2_t[:], lhs2_d[:])

            ident = consts.tile([P, P], bf16)
            make_identity(nc, ident[:])

            w21t0 = consts.tile([P, N], bf16)
            nc.sync.dma_start(w21t0[:], w21tb_d[0:P])
            w21t1 = consts.tile([P, N], bf16)
            nc.sync.dma_start(w21t1[:], w21tb_d[P : 2 * P])
            w21t2 = consts.tile([1, N], bf16)
            nc.sync.dma_start(w21t2[:], w21tb_d[2 * P : 2 * P + 1])

            eps_col = consts.tile([P, 1], f32)
            nc.vector.memset(eps_col[:], EPS)
            ones_row = consts.tile([1, N], bf16)
            nc.vector.memset(ones_row[:], 1.0)

            # ---------------- param p = relu(x @ W11.T + b11), f32 ----------
            p_ps = ps_k.tile([1, G_PER_CORE], f32, tag="small")
            nc.tensor.matmul(p_ps[:], w11t0[:], xta0[:], start=True, stop=False)
            nc.tensor.matmul(p_ps[:], w11t1[:], xta1[:], start=False, stop=False)
            nc.tensor.matmul(p_ps[:], w11t2[:], xta2[:], start=False, stop=True)
            p_sb = consts.tile([1, G_PER_CORE], f32)
            nc.scalar.activation(p_sb[:], p_ps[:], AF.Relu)
            nc.sync.dma_start(param_d[:], p_sb[:])

            # pmax = max(p, 1e-6); pinv = 1/pmax.  The sigmoid applies the
            # p multiply via its per-partition scale AP; the k matmul absorbs
            # 1/pmax via its stationary operand.
            pmax_sb = consts.tile([1, G_PER_CORE], f32)
            nc.vector.tensor_scalar_max(pmax_sb[:], p_sb[:], 1e-6)
            pinv_sb = consts.tile([1, G_PER_CORE], f32)
            nc.vector.reciprocal(pinv_sb[:], pmax_sb[:])
            psc_col = consts.tile([P, G_PER_CORE], f32)
            pinv16 = consts.tile([16, G_PER_CORE], f32)
            ones_col_f32 = consts.tile([1, P], f32)
            nc.vector.memset(ones_col_f32[:], 1.0)
            for g in range(G_PER_CORE):
                pc_ps = ps_k.tile([P, 1], f32, tag="small", name=f"pc_ps{g}")
                nc.tensor.matmul(
                    pc_ps[:], ones_col_f32[:], pmax_sb[:, g : g + 1],
                    start=True, stop=True,
                )
                nc.vector.tensor_copy(psc_col[:, g : g + 1], pc_ps[:])
                pi_ps = ps_k.tile([16, 1], f32, tag="small", name=f"pi_ps{g}")
                nc.tensor.matmul(
                    pi_ps[:], ones_col_f32[:, 0:16], pinv_sb[:, g : g + 1],
                    start=True, stop=True,
                )
                nc.vector.tensor_copy(pinv16[:, g : g + 1], pi_ps[:])

            # ------------- per-graph state -------------
            adj_sbs = [None] * G_PER_CORE
            outts = [None] * G_PER_CORE
            outtas = [None] * G_PER_CORE
            u_tiless = [[None] * NT for _ in range(G_PER_CORE)]

            def leaky(dst, src_ps, tmp, sl, on_dve):
                if on_dve:
                    nc.vector.tensor_scalar_mul(tmp[:, sl], src_ps[:], NEG)
                    nc.vector.tensor_tensor(dst[:, sl], src_ps[:], tmp[:, sl], op=OP.max)
                else:
                    nc.scalar.activation(dst[:, sl], src_ps[:], AF.Lrelu, alpha=NEG)

            def conv_chain(g, leaky_on_dve=False):
                # adjacency + invdeg loads
                adj_sb = adjp.tile([P, NT, N], bf16, name=f"adj_sb{g}", tag="adj")
                adj_view = adj_d[g].rearrange("(t p) d -> p t d", p=P)
                for s in range(NT):
                    nc.gpsimd.dma_start(adj_sb[:, s, :], adj_view[:, s, :])
                adj_sbs[g] = adj_sb
                invdeg_sb = work.tile([8, N], f32, name=f"invdeg{g}", tag="invdeg")
                nc.sync.dma_start(invdeg_sb[:], invdeg_d[g])
                yield

                # h1T [1, N] = leaky(x_g @ W21.T + b21)
                h1t = work.tile([1, N], bf16, name=f"h1t{g}", tag="h1t")
                h1tmp = work.tile([1, N], f32, name=f"h1tmp{g}", tag="h1tmp")
                for ck in range(2):
                    sl = ds(ck * 512, 512)
                    h1_ps = ps_big.tile([1, 512], f32, tag="big", name=f"h1ps{g}{ck}")
                    nc.tensor.matmul(h1_ps[:], xtb0[:, g : g + 1], w21t0[:, sl], start=True, stop=False)
                    nc.tensor.matmul(h1_ps[:], xtb1[:, g : g + 1], w21t1[:, sl], start=False, stop=False)
                    nc.tensor.matmul(h1_ps[:], xtb2[:, g : g + 1], w21t2[:, sl], start=False, stop=True)
                    leaky(h1t, h1_ps, h1tmp, sl, leaky_on_dve)

                yield
                # transpose h1T -> h1n [128, t]
                h1n = work.tile([P, NT], bf16, name=f"h1n{g}", tag="h1n")
                for t in range(NT):
                    tp_ps = ps_k.tile([P, 1], bf16, tag="small", name=f"tp{g}{t}")
                    nc.tensor.transpose(tp_ps[:], h1t[:, ts(t, P)], ident[0:1, 0:1])
                    nc.vector.tensor_copy(h1n[:, t : t + 1], tp_ps[:])

                yield
                # conv1 aggregation (s-major, per-512-chunk psums)
                rhs3 = work.tile([3, N], bf16, name=f"rhs3{g}", tag="rhs3")
                a1_c = [ps_big.tile([1, 512], f32, tag="big", name=f"a1c{g}{dc}") for dc in range(2)]
                for s in range(NT):
                    if s == NT // 2:
                        yield
                    for dc in range(2):
                        sl = ds(dc * 512, 512)
                        nc.tensor.matmul(
                            a1_c[dc][:], h1n[:, s : s + 1], adj_sb[:, s, sl],
                            start=(s == 0), stop=(s == NT - 1),
                        )
                # rhs3 rows: t1, h1t, ones
                for dc in range(2):
                    sl = ds(dc * 512, 512)
                    nc.vector.tensor_tensor(rhs3[0:1, sl], a1_c[dc][:], invdeg_sb[0:1, sl], op=OP.mult)
                nc.sync.dma_start(rhs3[1:2, :], h1t[:])
                nc.sync.dma_start(rhs3[2:3, :], ones_row[:])

                yield
                # conv1 linear (packed K=3) + leaky
                h2t = work.tile([8, N], bf16, name=f"h2t{g}", tag="h2t")
                h2tmp = work.tile([8, N], f32, name=f"h2tmp{g}", tag="h2tmp")
                for ck in range(2):
                    sl = ds(ck * 512, 512)
                    h2_ps = ps_big.tile([8, 512], f32, tag="big", name=f"h2ps{g}{ck}")
                    nc.tensor.matmul(h2_ps[:], lhs1_t[:], rhs3[:, sl], start=True, stop=True)
                    leaky(h2t, h2_ps, h2tmp, sl, leaky_on_dve)

                yield
                # transpose h2T -> h2n [128, (t, c)]
                h2n = work.tile([P, NT * 8], bf16, name=f"h2n{g}", tag="h2n")
                for t in range(NT):
                    tp2_ps = ps_k.tile([P, 8], bf16, tag="small", name=f"tp2{g}{t}")
                    nc.tensor.transpose(tp2_ps[:], h2t[:, ts(t, P)], ident[0:8, 0:8])
                    nc.vector.tensor_copy(h2n[:, t * 8 : (t + 1) * 8], tp2_ps[:])

                yield
                # conv2 aggregation
                rhs17 = work.tile([17, N], bf16, name=f"rhs17{g}", tag="rhs17")
                a2_c = [ps_big.tile([8, 512], f32, tag="big", name=f"a2c{g}{dc}") for dc in range(2)]
                for s in range(NT):
                    if s == NT // 2:
                        yield
                    for dc in range(2):
                        sl = ds(dc * 512, 512)
                        nc.tensor.matmul(
                            a2_c[dc][:], h2n[:, s * 8 : (s + 1) * 8], adj_sb[:, s, sl],
                            start=(s == 0), stop=(s == NT - 1),
                        )
                yield
                # rhs17 rows: t2 (8), h2t (8), ones
                for dc in range(2):
                    sl = ds(dc * 512, 512)
                    nc.vector.tensor_tensor(rhs17[0:8, sl], a2_c[dc][:], invdeg_sb[:, sl], op=OP.mult)
                nc.sync.dma_start(rhs17[8:16, :], h2t[:])
                nc.sync.dma_start(rhs17[16:17, :], ones_row[:])

                yield
                # conv2 linear (packed K=17) + leaky -> outT [16, N]
                outt = work.tile([16, N], bf16, name=f"outt{g}", tag="outt")
                otmp = work.tile([16, N], f32, name=f"otmp{g}", tag="otmp")
                for ck in range(2):
                    sl = ds(ck * 512, 512)
                    o_ps = ps_big.tile([16, 512], f32, tag="big", name=f"ops{g}{ck}")
                    nc.tensor.matmul(o_ps[:], lhs2_t[:], rhs17[:, sl], start=True, stop=True)
                    leaky(outt, o_ps, otmp, sl, leaky_on_dve)
                outts[g] = outt

                # scaled outT for the k matmul: outt_a = outt / pmax
                outt_a = work.tile([16, N], bf16, name=f"outta{g}", tag="outta")
                nc.vector.tensor_scalar_mul(outt_a[:], outt[:], pinv16[:, g : g + 1])
                outtas[g] = outt_a

                # phase A: tln[t] = ln(adj_row_t + eps), bf16
                for t in range(NT):
                    yield
                    tln = up.tile([P, N], bf16, tag="u", name=f"tln{g}{t}")
                    nc.scalar.activation(tln[:], adj_sb[:, t, :], AF.Ln, bias=eps_col[:])
                    u_tiless[g][t] = tln

            def k_unit(g, t):
                outt = outts[g]
                conn_sb = outp.tile([P, N], f32, name=f"conn{g}{t}", tag="conn")
                v = ewp.tile([P, N], f32, tag="v", name=f"v{g}{t}", bufs=4)
                for mc in range(2):
                    msl = ds(mc * 512, 512)
                    k_ps = ps_k.tile([P, 512], f32, tag="kk", bufs=3, name=f"kps{g}{t}{mc}")
                    nc.tensor.matmul(
                        k_ps[:], outtas[g][:, ts(t, P)], outt[:, msl],
                        start=True, stop=True,
                    )
                    nc.vector.tensor_tensor(v[:, msl], u_tiless[g][t][:, msl], k_ps[:], op=OP.add)
                nc.scalar.activation(conn_sb[:], v[:], AF.Sigmoid, scale=psc_col[:, g : g + 1])
                nc.scalar.dma_start(conn_d[g, ts(t, P), :], conn_sb[:])

            # emit order == per-engine execution order: conv(0) first (its adj
            # stream and agg chain are the critical path), then k(0) units
            # interleaved with conv(1) stages, then k(1).
            for _ in conv_chain(0, leaky_on_dve=True):
                pass
            t0 = 0
            for _ in conv_chain(1, leaky_on_dve=True):
                if t0 < NT:
                    k_unit(0, t0)
                    t0 += 1
            while t0 < NT:
                k_unit(0, t0)
                t0 += 1
            for t in range(NT):
                k_unit(1, t)

    nc.compile()
    return nc


def _get_nc():
    if "nc" not in _CACHE:
        _CACHE["nc"] = _build_nc()
    return _CACHE["nc"]


def kernel(
    x, edge_index, edge_attr, batch,
    W11, b11, W21, b21,
    Wrel1, brel1, Wroot1, Wrel2, brel2, Wroot2,
):
    import ml_dtypes

    from concourse.bass_utils import run_bass_kernel_spmd

    bf = ml_dtypes.bfloat16

    x = np.asarray(x, dtype=np.float32)
    edge_attr = np.asarray(edge_attr, dtype=np.float32)
    src = np.asarray(edge_index[0], dtype=np.int64)
    dst = np.asarray(edge_index[1], dtype=np.int64)

    # ---- host index preprocessing ----
    g_of_edge = src // N
    flat = g_of_edge * (N * N) + (src % N) * N + (dst % N)
    adj = (
        np.bincount(flat, weights=edge_attr.astype(np.float64), minlength=B * N * N)
        .astype(np.float32)
        .reshape(B, N, N)
    )
    adj_bf = adj.astype(bf)
    deg = np.bincount(dst, minlength=B * N).reshape(B, N).astype(np.float32)
    invdeg = (1.0 / np.maximum(deg, 1.0)).astype(np.float32)
    invdeg_rep = np.ascontiguousarray(
        np.broadcast_to(invdeg[:, None, :], (B, 8, N)), dtype=np.float32
    )

    # ---- weight layout prep ----
    ones_b = np.ones((1, B), np.float32)
    xta = np.concatenate([x.T, ones_b], axis=0).astype(np.float32)  # [257, 16]
    w21tb = np.concatenate(
        [np.asarray(W21, np.float32).T, np.asarray(b21, np.float32)[None, :]], axis=0
    ).astype(bf)  # [257, 1024]
    w11tb = np.concatenate(
        [np.asarray(W11, np.float32).T, np.asarray(b11, np.float32)[None, :]], axis=0
    )  # [257, 1]
    lhs1 = np.stack(
        [
            np.asarray(Wrel1, np.float32)[:, 0],
            np.asarray(Wroot1, np.float32)[:, 0],
            np.asarray(brel1, np.float32),
        ],
        axis=0,
    ).astype(bf)  # [3, 8]
    lhs2 = np.concatenate(
        [
            np.asarray(Wrel2, np.float32).T,
            np.asarray(Wroot2, np.float32).T,
            np.asarray(brel2, np.float32)[None, :],
        ],
        axis=0,
    ).astype(bf)  # [17, 16]

    nc = _get_nc()
    in_maps = []
    for c in range(N_CORES):
        gs = slice(c * G_PER_CORE, (c + 1) * G_PER_CORE)
        in_maps.append(
            {
                "adj": np.ascontiguousarray(adj_bf[gs]),
                "xta": np.ascontiguousarray(xta[:, gs]),
                "xtab": np.ascontiguousarray(xta[:, gs]).astype(bf),
                "w21tb": w21tb,
                "w11tb": w11tb,
                "lhs1": lhs1,
                "lhs2": lhs2,
                "invdeg": np.ascontiguousarray(invdeg_rep[gs]),
            }
        )

    res = run_bass_kernel_spmd(nc, in_maps, list(range(N_CORES)), **_CACHE.get("run_kwargs", {}))
    _CACHE["last_result"] = res

    conn = np.concatenate([res.results[c]["conn"] for c in range(N_CORES)], axis=0)
    param = np.concatenate(
        [res.results[c]["param"][0] for c in range(N_CORES)], axis=0
    ).reshape(B, 1)
    return conn, param


# revision 35
# speedup vs baseline: 1.0823x; 1.0823x over previous
"""Trainium2 Bass kernel for nn_BRIDGEDecoder (gnn_message_passing).

Strategy: data-parallel over the 16 graphs, 2 graphs per NeuronCore.
Host does index preprocessing only (dense adjacency + in-degree histograms
via bincount, weight layout prep); the device does all float NN compute:
  h1 = leaky(x @ W21.T + b21)            (per-graph [1,1024] matmul)
  agg = (adj.T @ h) / deg                (dense matmuls against SBUF adjacency)
  h2 = leaky([Wrel1|Wroot1|b] @ [t1;h1;1])   (packed K=3 matmul, channel-major)
  out = leaky([Wrel2|Wroot2|b] @ [t2;h2;1])  (packed K=17 matmul)
  k = (out/pmax).T @ out per n-tile      (PE, K=16, bf16)
  conn = sigmoid(pmax * (ln(adj+1e-10) + k/pmax))   with pmax = max(p, 1e-6):
         the per-graph p multiply is folded into the Sigmoid's per-partition
         scale AP, and 1/pmax into the k matmul's stationary operand.
  param = relu(x @ W11.T + b11)          (f32 path)

bf16 is used for the adjacency stream and matmul operands (PE runs 1 cyc/col
at bf16 vs 2 at fp32, and adjacency DMA halves); accumulation stays fp32 in
PSUM, and the sigmoid chain runs fp32. The param path is kept fp32.
Emit order is per-engine execution order: conv(0) first, then k(0) units
interleaved with conv(1) stages (graph 1's leaky runs on DVE so the ACT
queue during the k phase stays pure Sigmoid), then k(1).
"""

import sys

if "/opt/trn_rl_repo" not in sys.path:
    sys.path.insert(0, "/opt/trn_rl_repo")

import numpy as np

B, N, F = 16, 1024, 256
P = 128
NT = N // P  # 8 n-tiles per graph
G_PER_CORE = 2
N_CORES = 8
EPS = 1e-10
NEG = 0.01

_CACHE = {}


def _build_nc():
    import concourse.mybir as mybir
    import concourse.tile as tile
    from concourse import bacc
    from concourse.bass import ds, ts
    from concourse.masks import make_identity

    f32 = mybir.dt.float32
    bf16 = mybir.dt.bfloat16
    AF = mybir.ActivationFunctionType
    OP = mybir.AluOpType

    nc = bacc.Bacc("TRN2", target_bir_lowering=False, debug=False)

    adj_d = nc.dram_tensor("adj", [G_PER_CORE, N, N], bf16, kind="ExternalInput")
    xta_d = nc.dram_tensor("xta", [F + 1, G_PER_CORE], f32, kind="ExternalInput")
    xtab_d = nc.dram_tensor("xtab", [F + 1, G_PER_CORE], bf16, kind="ExternalInput")
    w21tb_d = nc.dram_tensor("w21tb", [F + 1, N], bf16, kind="ExternalInput")
    w11tb_d = nc.dram_tensor("w11tb", [F + 1, 1], f32, kind="ExternalInput")
    lhs1_d = nc.dram_tensor("lhs1", [3, 8], bf16, kind="ExternalInput")
    lhs2_d = nc.dram_tensor("lhs2", [17, 16], bf16, kind="ExternalInput")
    invdeg_d = nc.dram_tensor("invdeg", [G_PER_CORE, 8, N], f32, kind="ExternalInput")

    conn_d = nc.dram_tensor("conn", [G_PER_CORE, N, N], f32, kind="ExternalOutput")
    param_d = nc.dram_tensor("param", [1, G_PER_CORE], f32, kind="ExternalOutput")

    with tile.TileContext(nc) as tc:
        with (
            tc.tile_pool(name="consts", bufs=1) as consts,
            tc.tile_pool(name="adjp", bufs=2) as adjp,
            tc.tile_pool(name="work", bufs=2) as work,
            tc.tile_pool(name="ew", bufs=3) as ewp,
            tc.tile_pool(name="up", bufs=17) as up,
            tc.tile_pool(name="outp", bufs=3) as outp,
            tc.tile_pool(name="ps_big", bufs=3, space="PSUM") as ps_big,
            tc.tile_pool(name="ps_k", bufs=2, space="PSUM") as ps_k,
        ):
            # ---------------- constants (small tensors first: param path) ----
            xta0 = consts.tile([P, G_PER_CORE], f32)
            nc.sync.dma_start(xta0[:], xta_d[0:P])
            xta1 = consts.tile([P, G_PER_CORE], f32)
            nc.sync.dma_start(xta1[:], xta_d[P : 2 * P])
            xta2 = consts.tile([1, G_PER_CORE], f32)
            nc.sync.dma_start(xta2[:], xta_d[2 * P : 2 * P + 1])

            w11t0 = consts.tile([P, 1], f32)
            nc.sync.dma_start(w11t0[:], w11tb_d[0:P])
            w11t1 = consts.tile([P, 1], f32)
            nc.sync.dma_start(w11t1[:], w11tb_d[P : 2 * P])
            w11t2 = consts.tile([1, 1], f32)
            nc.sync.dma_start(w11t2[:], w11tb_d[2 * P : 2 * P + 1])

            xtb0 = consts.tile([P, G_PER_CORE], bf16)
            nc.sync.dma_start(xtb0[:], xtab_d[0:P])
            xtb1 = consts.tile([P, G_PER_CORE], bf16)
            nc.sync.dma_start(xtb1[:], xtab_d[P : 2 * P])
            xtb2 = consts.tile([1, G_PER_CORE], bf16)
            nc.sync.dma_start(xtb2[:], xtab_d[2 * P : 2 * P + 1])

            lhs1_t = consts.tile([3, 8], bf16)  # [Wrel1; Wroot1; brel1]
            nc.sync.dma_start(lhs1_t[:], lhs1_d[:])
            lhs2_t = consts.tile([17, 16], bf16)  # [Wrel2.T; Wroot2.T; brel2]
            nc.sync.dma_start(lhs2_t[:], lhs2_d[:])

            ident = consts.tile([P, P], bf16)
            make_identity(nc, ident[:])

            w21t0 = consts.tile([P, N], bf16)
            nc.sync.dma_start(w21t0[:], w21tb_d[0:P])
            w21t1 = consts.tile([P, N], bf16)
            nc.sync.dma_start(w21t1[:], w21tb_d[P : 2 * P])
            w21t2 = consts.tile([1, N], bf16)
            nc.sync.dma_start(w21t2[:], w21tb_d[2 * P : 2 * P + 1])

            eps_col = consts.tile([P, 1], f32)
            nc.vector.memset(eps_col[:], EPS)
            ones_row = consts.tile([1, N], bf16)
            nc.vector.memset(ones_row[:], 1.0)

            # ---------------- param p = relu(x @ W11.T + b11), f32 ----------
            p_ps = ps_k.tile([1, G_PER_CORE], f32, tag="small")
            nc.tensor.matmul(p_ps[:], w11t0[:], xta0[:], start=True, stop=False)
            nc.tensor.matmul(p_ps[:], w11t1[:], xta1[:], start=False, stop=False)
            nc.tensor.matmul(p_ps[:], w11t2[:], xta2[:], start=False, stop=True)
            p_sb = consts.tile([1, G_PER_CORE], f32)
            nc.scalar.activation(p_sb[:], p_ps[:], AF.Relu)
            nc.sync.dma_start(param_d[:], p_sb[:])

            # pmax = max(p, 1e-6); pinv = 1/pmax.  The sigmoid applies the
            # p multiply via its per-partition scale AP; the k matmul absorbs
            # 1/pmax via its stationary operand.
            pmax_sb = consts.tile([1, G_PER_CORE], f32)
            nc.vector.tensor_scalar_max(pmax_sb[:], p_sb[:], 1e-6)
            pinv_sb = consts.tile([1, G_PER_CORE], f32)
            nc.vector.reciprocal(pinv_sb[:], pmax_sb[:])
            psc_col = consts.tile([P, G_PER_CORE], f32)
            pinv16 = consts.tile([16, G_PER_CORE], f32)
            ones_col_f32 = consts.tile([1, P], f32)
            nc.vector.memset(ones_col_f32[:], 1.0)
            for g in range(G_PER_CORE):
                pc_ps = ps_k.tile([P, 1], f32, tag="small", name=f"pc_ps{g}")
                nc.tensor.matmul(
                    pc_ps[:], ones_col_f32[:], pmax_sb[:, g : g + 1],
                    start=True, stop=True,
                )
                nc.vector.tensor_copy(psc_col[:, g : g + 1], pc_ps[:])
                pi_ps = ps_k.tile([16, 1], f32, tag="small", name=f"pi_ps{g}")
                nc.tensor.matmul(
                    pi_ps[:], ones_col_f32[:, 0:16], pinv_sb[:, g : g + 1],
                    start=True, stop=True,
                )
                nc.vector.tensor_copy(pinv16[:, g : g + 1], pi_ps[:])

            # ------------- per-graph state -------------
            adj_sbs = [None] * G_PER_CORE
            outts = [None] * G_PER_CORE
            outtas = [None] * G_PER_CORE
            u_tiless = [[None] * NT for _ in range(G_PER_CORE)]

            def leaky(dst, src_ps, tmp, sl, on_dve):
                if on_dve:
                    nc.vector.tensor_scalar_mul(tmp[:, sl], src_ps[:], NEG)
                    nc.vector.tensor_tensor(dst[:, sl], src_ps[:], tmp[:, sl], op=OP.max)
                else:
                    nc.scalar.activation(dst[:, sl], src_ps[:], AF.Lrelu, alpha=NEG)

            def conv_chain(g, leaky_on_dve=False):
                # adjacency + invdeg loads
                adj_sb = adjp.tile([P, NT, N], bf16, name=f"adj_sb{g}", tag="adj")
                adj_view = adj_d[g].rearrange("(t p) d -> p t d", p=P)
                for s in range(NT):
                    nc.gpsimd.dma_start(adj_sb[:, s, :], adj_view[:, s, :])
                adj_sbs[g] = adj_sb
                invdeg_sb = work.tile([8, N], f32, name=f"invdeg{g}", tag="invdeg")
                nc.sync.dma_start(invdeg_sb[:], invdeg_d[g])
                yield

                # h1T [1, N] = leaky(x_g @ W21.T + b21)
                h1t = work.tile([1, N], bf16, name=f"h1t{g}", tag="h1t")
                h1tmp = work.tile([1, N], f32, name=f"h1tmp{g}", tag="h1tmp")
                for ck in range(2):
                    sl = ds(ck * 512, 512)
                    h1_ps = ps_big.tile([1, 512], f32, tag="big", name=f"h1ps{g}{ck}")
                    nc.tensor.matmul(h1_ps[:], xtb0[:, g : g + 1], w21t0[:, sl], start=True, stop=False)
                    nc.tensor.matmul(h1_ps[:], xtb1[:, g : g + 1], w21t1[:, sl], start=False, stop=False)
                    nc.tensor.matmul(h1_ps[:], xtb2[:, g : g + 1], w21t2[:, sl], start=False, stop=True)
                    leaky(h1t, h1_ps, h1tmp, sl, leaky_on_dve)

                yield
                # transpose h1T -> h1n [128, t]
                h1n = work.tile([P, NT], bf16, name=f"h1n{g}", tag="h1n")
                for t in range(NT):
                    tp_ps = ps_k.tile([P, 1], bf16, tag="small", name=f"tp{g}{t}")
                    nc.tensor.transpose(tp_ps[:], h1t[:, ts(t, P)], ident[0:1, 0:1])
                    nc.vector.tensor_copy(h1n[:, t : t + 1], tp_ps[:])

                yield
                # conv1 aggregation (s-major, per-512-chunk psums)
                rhs3 = work.tile([3, N], bf16, name=f"rhs3{g}", tag="rhs3")
                a1_c = [ps_big.tile([1, 512], f32, tag="big", name=f"a1c{g}{dc}") for dc in range(2)]
                for s in range(NT):
                    if s == NT // 2:
                        yield
                    for dc in range(2):
                        sl = ds(dc * 512, 512)
                        nc.tensor.matmul(
                            a1_c[dc][:], h1n[:, s : s + 1], adj_sb[:, s, sl],
                            start=(s == 0), stop=(s == NT - 1),
                        )
                # rhs3 rows: t1, h1t, ones
                for dc in range(2):
                    sl = ds(dc * 512, 512)
                    nc.vector.tensor_tensor(rhs3[0:1, sl], a1_c[dc][:], invdeg_sb[0:1, sl], op=OP.mult)
                nc.sync.dma_start(rhs3[1:2, :], h1t[:])
                nc.sync.dma_start(rhs3[2:3, :], ones_row[:])

                yield
                # conv1 linear (packed K=3) + leaky
                h2t = work.tile([8, N], bf16, name=f"h2t{g}", tag="h2t")
                h2tmp = work.tile([8, N], f32, name=f"h2tmp{g}", tag="h2tmp")
                for ck in range(2):
                    sl = ds(ck * 512, 512)
                    h2_ps = ps_big.tile([8, 512], f32, tag="big", name=f"h2ps{g}{ck}")
                    nc.tensor.matmul(h2_ps[:], lhs1_t[:], rhs3[:, sl], start=True, stop=True)
                    leaky(h2t, h2_ps, h2tmp, sl, leaky_on_dve)

                yield
                # transpose h2T -> h2n [128, (t, c)]
                h2n = work.tile([P, NT * 8], bf16, name=f"h2n{g}", tag="h2n")
                for t in range(NT):
                    tp2_ps = ps_k.tile([P, 8], bf16, tag="small", name=f"tp2{g}{t}")
                    nc.tensor.transpose(tp2_ps[:], h2t[:, ts(t, P)], ident[0:8, 0:8])
                    nc.vector.tensor_copy(h2n[:, t * 8 : (t + 1) * 8], tp2_ps[:])

                yield
                # conv2 aggregation
                rhs17 = work.tile([17, N], bf16, name=f"rhs17{g}", tag="rhs17")
                a2_c = [ps_big.tile([8, 512], f32, tag="big", name=f"a2c{g}{dc}") for dc in range(2)]
                for s in range(NT):
                    if s == NT // 2:
                        yield
                    for dc in range(2):
                        sl = ds(dc * 512, 512)
                        nc.tensor.matmul(
                            a2_c[dc][:], h2n[:, s * 8 : (s + 1) * 8], adj_sb[:, s, sl],
                            start=(s == 0), stop=(s == NT - 1),
                        )
                yield
                # rhs17 rows: t2 (8), h2t (8), ones
                for dc in range(2):
                    sl = ds(dc * 512, 512)
                    nc.vector.tensor_tensor(rhs17[0:8, sl], a2_c[dc][:], invdeg_sb[:, sl], op=OP.mult)
                nc.sync.dma_start(rhs17[8:16, :], h2t[:])
                nc.sync.dma_start(rhs17[16:17, :], ones_row[:])

                yield
                # conv2 linear (packed K=17) + leaky -> outT [16, N]
                outt = work.tile([16, N], bf16, name=f"outt{g}", tag="outt")
                otmp = work.tile([16, N], f32, name=f"otmp{g}", tag="otmp")
                for ck in range(2):
                    sl = ds(ck * 512, 512)
                    o_ps = ps_big.tile([16, 512], f32, tag="big", name=f"ops{g}{ck}")
                    nc.tensor.matmul(o_ps[:], lhs2_t[:], rhs17[:, sl], start=True, stop=True)
                    leaky(outt, o_ps, otmp, sl, leaky_on_dve)
                outts[g] = outt

                # scaled outT for the k matmul: outt_a = outt / pmax
                outt_a = work.tile([16, N], bf16, name=f"outta{g}", tag="outta")
                nc.vector.tensor_scalar_mul(outt_a[:], outt[:], pinv16[:, g : g + 1])
                outtas[g] = outt_a

                # phase A: tln[t] = ln(adj_row_t + eps), bf16
                for t in range(NT):
                    yield
                    tln = up.tile([P, N], bf16, tag="u", name=f"tln{g}{t}")
                    nc.scalar.activation(tln[:], adj_sb[:, t, :], AF.Ln, bias=eps_col[:])
                    u_tiless[g][t] = tln

            def k_unit(g, t):
                outt = outts[g]
                conn_sb = outp.tile([P, N], f32, name=f"conn{g}{t}", tag="conn")
                v = ewp.tile([P, N], f32, tag="v", name=f"v{g}{t}", bufs=4)
                for mc in range(2):
                    msl = ds(mc * 512, 512)
                    k_ps = ps_k.tile([P, 512], f32, tag="kk", bufs=3, name=f"kps{g}{t}{mc}")
                    nc.tensor.matmul(
                        k_ps[:], outtas[g][:, ts(t, P)], outt[:, msl],
                        start=True, stop=True,
                    )
                    nc.vector.tensor_tensor(v[:, msl], u_tiless[g][t][:, msl], k_ps[:], op=OP.add)
                nc.scalar.activation(conn_sb[:], v[:], AF.Sigmoid, scale=psc_col[:, g : g + 1])
                nc.scalar.dma_start(conn_d[g, ts(t, P), :], conn_sb[:])

            # emit order == per-engine execution order: conv(0) first (its adj
            # stream and agg chain are the critical path), then k(0) units
            # interleaved with conv(1) stages, then k(1).
            for _ in conv_chain(0, leaky_on_dve=False):
                pass
            t0 = 0
            for _ in conv_chain(1, leaky_on_dve=True):
                if t0 < NT:
                    k_unit(0, t0)
                    t0 += 1
            while t0 < NT:
                k_unit(0, t0)
                t0 += 1
            for t in range(NT):
                k_unit(1, t)

    nc.compile()
    return nc


def _get_nc():
    if "nc" not in _CACHE:
        _CACHE["nc"] = _build_nc()
    return _CACHE["nc"]


def kernel(
    x, edge_index, edge_attr, batch,
    W11, b11, W21, b21,
    Wrel1, brel1, Wroot1, Wrel2, brel2, Wroot2,
):
    import ml_dtypes

    from concourse.bass_utils import run_bass_kernel_spmd

    bf = ml_dtypes.bfloat16

    x = np.asarray(x, dtype=np.float32)
    edge_attr = np.asarray(edge_attr, dtype=np.float32)
    src = np.asarray(edge_index[0], dtype=np.int64)
    dst = np.asarray(edge_index[1], dtype=np.int64)

    # ---- host index preprocessing ----
    g_of_edge = src // N
    flat = g_of_edge * (N * N) + (src % N) * N + (dst % N)
    adj = (
        np.bincount(flat, weights=edge_attr.astype(np.float64), minlength=B * N * N)
        .astype(np.float32)
        .reshape(B, N, N)
    )
    adj_bf = adj.astype(bf)
    deg = np.bincount(dst, minlength=B * N).reshape(B, N).astype(np.float32)
    invdeg = (1.0 / np.maximum(deg, 1.0)).astype(np.float32)
    invdeg_rep = np.ascontiguousarray(
        np.broadcast_to(invdeg[:, None, :], (B, 8, N)), dtype=np.float32
    )

    # ---- weight layout prep ----
    ones_b = np.ones((1, B), np.float32)
    xta = np.concatenate([x.T, ones_b], axis=0).astype(np.float32)  # [257, 16]
    w21tb = np.concatenate(
        [np.asarray(W21, np.float32).T, np.asarray(b21, np.float32)[None, :]], axis=0
    ).astype(bf)  # [257, 1024]
    w11tb = np.concatenate(
        [np.asarray(W11, np.float32).T, np.asarray(b11, np.float32)[None, :]], axis=0
    )  # [257, 1]
    lhs1 = np.stack(
        [
            np.asarray(Wrel1, np.float32)[:, 0],
            np.asarray(Wroot1, np.float32)[:, 0],
            np.asarray(brel1, np.float32),
        ],
        axis=0,
    ).astype(bf)  # [3, 8]
    lhs2 = np.concatenate(
        [
            np.asarray(Wrel2, np.float32).T,
            np.asarray(Wroot2, np.float32).T,
            np.asarray(brel2, np.float32)[None, :],
        ],
        axis=0,
    ).astype(bf)  # [17, 16]

    nc = _get_nc()
    in_maps = []
    for c in range(N_CORES):
        gs = slice(c * G_PER_CORE, (c + 1) * G_PER_CORE)
        in_maps.append(
            {
                "adj": np.ascontiguousarray(adj_bf[gs]),
                "xta": np.ascontiguousarray(xta[:, gs]),
                "xtab": np.ascontiguousarray(xta[:, gs]).astype(bf),
                "w21tb": w21tb,
                "w11tb": w11tb,
                "lhs1": lhs1,
                "lhs2": lhs2,
                "invdeg": np.ascontiguousarray(invdeg_rep[gs]),
            }
        )

    res = run_bass_kernel_spmd(nc, in_maps, list(range(N_CORES)), **_CACHE.get("run_kwargs", {}))
    _CACHE["last_result"] = res

    conn = np.concatenate([res.results[c]["conn"] for c in range(N_CORES)], axis=0)
    param = np.concatenate(
        [res.results[c]["param"][0] for c in range(N_CORES)], axis=0
    ).reshape(B, 1)
    return conn, param
